# revision 18
# baseline (speedup 1.0000x reference)
"""Trainium2 Bass kernel for a dense transformer block (RMSNorm->MHA->res, RMSNorm->SwiGLU-FFN->res).

Sharding over 8 NeuronCores: fsdp=2 (batch) x tp=4 (attention heads / FFN hidden).
Core 4*b + t handles batch b with TP-rank t (heads 4t..4t+3, FFN hidden cols 2048t..2048(t+1)).

All matmul operands are bf16 (full PE rate, FWL weight loads, half the SBUF/DMA
bytes); accumulation and the residual stream stay fp32.  On-device activations are
feature-major ([features, rows]) so matmuls chain without transposes.  Streamed
weights are pre-arranged on the host into per-partition-contiguous [128, X]
layouts so every weight DMA is a single >=2KB descriptor per partition.

Collective plan (per 4-core TP group, all payloads bf16 -- CCE adds in bf16):
  AG1(qb): AllGather of per-rank attention-head outputs ao [512(hd),512] for
           q-block qb -> [2048(hd),512].  Each rank then computes the FULL
           out-projection for its 512-row E-shard locally (same FLOPs as the
           head-sharded projection; an AllGather costs half a ReduceScatter on
           the wire and removes partial-sum staging).
  AG2(h):  AllGather of raw x2 E-shards [512,1024] -> [2048,1024] per row-half.
           rms2 runs locally AFTER the gather (no mean-square AllReduce).
  RS2(g):  ReduceScatter of FFN down partials [2048,512] -> [512,512] per
           512-row group.
Emission order gives every collective >=1 compute phase of slack before its
first consumer's DMA is issued, so engine sequencers never block on collective
semaphores.  Final output per core: [its 512 E-features, 2048 rows].
"""

import numpy as np

EMBED = 2048
HEADS = 16
HEAD_DIM = 128
FF_HID = 8192
BATCH = 2
SEQ = 2048
EPS = 1e-6

N_CORES = 8
TP = 4
GROUPS = [[0, 1, 2, 3], [4, 5, 6, 7]]
H_LOC = HEADS // TP          # 4 heads per core
F_LOC = FF_HID // TP         # 2048 ffn-hidden per core
ROWS = SEQ                   # 2048 rows per batch
ROWS_T = ROWS // TP          # 512 rows per tp-rank (E-shard size)
P = 128
NE = EMBED // P              # 16 embed chunks
NF = F_LOC // P              # 16 ffn chunks
NR = ROWS // P               # 16 row chunks
QB = 512                     # q-block / phase row count / matmul moving size
NQB = ROWS // QB             # 4 phases
RH = 1024                    # ffn row-half (AG2 granularity)
HD = H_LOC * HEAD_DIM        # 512 local head-dim rows
NHD = HEADS * HEAD_DIM // P  # 16 gathered hd chunks
INV_SQRT_D = float(1.0 / np.sqrt(HEAD_DIM))
# fp8(e4m3, max 240) FFN weight pre-scales keep the tiny (std~0.01) weights out
# of the subnormal range; the gelu input scale and the down-stage copy undo them.
WG_SCALE = 64.0
WU_SCALE = 16.0
WD_SCALE = 64.0

_NC_CACHE = {}


def build_kernel():
    import concourse.mybir as mybir
    import concourse.tile as tile
    from concourse import bacc

    f32 = mybir.dt.float32
    bf16 = mybir.dt.bfloat16

    nc = bacc.Bacc("TRN2", target_bir_lowering=False, debug=False, num_devices=N_CORES)

    io = {}
    io["xb"] = nc.dram_tensor("xb", [EMBED, ROWS], bf16, kind="ExternalInput").ap()
    io["xte"] = nc.dram_tensor("xte", [ROWS_T, ROWS], f32, kind="ExternalInput").ap()
    io["wq"] = nc.dram_tensor("wq", [H_LOC, P, NE * HEAD_DIM], bf16, kind="ExternalInput").ap()
    io["wk"] = nc.dram_tensor("wk", [H_LOC, P, NE * HEAD_DIM], bf16, kind="ExternalInput").ap()
    io["wv"] = nc.dram_tensor("wv", [P, NE * HD], bf16, kind="ExternalInput").ap()
    io["wout2"] = nc.dram_tensor("wout2", [P, NHD * ROWS_T], bf16, kind="ExternalInput").ap()
    f8 = mybir.dt.float8e4
    io["wg"] = nc.dram_tensor("wg", [NF, P, NE * P], f8, kind="ExternalInput").ap()
    io["wu"] = nc.dram_tensor("wu", [NF, P, NE * P], f8, kind="ExternalInput").ap()
    io["wd"] = nc.dram_tensor("wd", [NE, P, NF * P], f8, kind="ExternalInput").ap()
    io["masks"] = nc.dram_tensor("masks", [P, QB + 3 * P], bf16, kind="ExternalInput").ap()
    io["ones"] = nc.dram_tensor("ones", [P, 1], bf16, kind="ExternalInput").ap()
    io["out"] = nc.dram_tensor("out", [ROWS_T, ROWS], f32, kind="ExternalOutput").ap()

    with tile.TileContext(nc) as tc:
        _emit(tc, nc, io)
    nc.compile()
    return nc


def _emit(tc, nc, io):
    from contextlib import ExitStack

    import concourse.mybir as mybir

    f32 = mybir.dt.float32
    bf16 = mybir.dt.bfloat16
    f8 = mybir.dt.float8e4
    DR = mybir.MatmulPerfMode.DoubleRow
    AF = mybir.ActivationFunctionType

    xb_in, xte, wq, wk, wv = io["xb"], io["xte"], io["wq"], io["wk"], io["wv"]
    ones_in = io["ones"]
    wout2, wg, wu, wd, masks = io["wout2"], io["wg"], io["wu"], io["wd"], io["masks"]
    out_ext = io["out"]

    def r3(ap2d, cols=None):
        """[(o p), q] dram view -> [p, o, q]; optionally slice columns first."""
        v = ap2d if cols is None else ap2d[:, cols]
        return v.rearrange("(o p) q -> p o q", p=P)

    ctx = ExitStack()
    with ctx:
        consts = ctx.enter_context(tc.tile_pool(name="consts", bufs=1))
        dram = ctx.enter_context(tc.tile_pool(name="dram", bufs=1, space="DRAM"))
        psum = ctx.enter_context(tc.tile_pool(name="psum", bufs=2, space="PSUM"))

        ones_sb = consts.tile([P, 1], bf16)
        nc.sync.dma_start(ones_sb[:], ones_in[:])
        eps_sb = consts.tile([1, 1], f32)
        nc.vector.memset(eps_sb[:], EPS)
        mask_sb = consts.tile([P, QB + 3 * P], bf16)
        nc.sync.dma_start(mask_sb[:], masks[:])
        wo_sb = consts.tile([P, NHD, ROWS_T], bf16)  # DMA deferred off the startup path

        # one dram tile per collective slice — a shared tensor would add false
        # whole-tensor deps (e.g. an ag2_out[0] reader waiting on AG2(1)'s write)
        ag1_in = [dram.tile([HD, QB], bf16, name=f"ag1i{i}") for i in range(NQB)]
        ag1_out = [dram.tile([HEADS * HEAD_DIM, QB], bf16, name=f"ag1o{i}") for i in range(NQB)]
        ag2_in = [dram.tile([ROWS_T, RH], bf16, name=f"ag2i{i}") for i in range(2)]
        ag2_out = [dram.tile([EMBED, RH], bf16, name=f"ag2o{i}") for i in range(2)]
        rs2_in = [dram.tile([EMBED, QB], bf16, name=f"rs2i{i}") for i in range(NQB)]
        rs2_out = [dram.tile([ROWS_T, QB], bf16, name=f"rs2o{i}") for i in range(NQB)]
        x2_scr = dram.tile([ROWS_T, ROWS], f32)

        with tc.tile_pool(name="s12", bufs=1) as s12:
            k_store = s12.tile([P, H_LOC, ROWS], bf16)
            v_store = s12.tile([P, NR, H_LOC, HEAD_DIM], bf16)

            tiles = {}

            def big16(name):
                """3-slot rotation shared by xq(0..3), ago(0..3) and n2(0..3) —
                16KB/partition tiles whose live ranges interleave exactly 3-deep."""
                return s12.tile([P, NE, QB], bf16, tag="big16", bufs=3, name=name)

            # ---------- stage 1+2 pieces ----------
            def emit_xb_dma(qb):
                xq = big16(f"xq{qb}")
                nc.sync.dma_start(xq[:], r3(xb_in, slice(qb * QB, (qb + 1) * QB)))
                tiles[("xq", qb)] = xq

            def emit_sq_ms(qb):
                ms = psum.tile([1, QB], f32, tag="acc", bufs=2, name=f"ms{qb}")
                xq = tiles[("xq", qb)]
                for e in range(NE):
                    sq = s12.tile([P, QB], bf16, tag="sq", bufs=2)
                    nc.vector.tensor_mul(sq[:], xq[:, e, :], xq[:, e, :])
                    nc.tensor.matmul(ms[:], ones_sb[:], sq[:],
                                     start=(e == 0), stop=(e == NE - 1))
                tiles[("ms", qb)] = ms

            def emit_norm_tail(qb):
                ms = tiles.pop(("ms", qb))
                rsq = s12.tile([1, QB], f32, tag="rsq", bufs=1)
                nc.scalar.activation(rsq[:], ms[:], AF.Sqrt, bias=eps_sb[:], scale=1.0 / EMBED)
                rsq_i = s12.tile([1, QB], f32, tag="rsqi", bufs=1)
                nc.vector.reciprocal(rsq_i[:], rsq[:])
                bc = s12.tile([P, QB], f32, tag="bc", bufs=2)
                nc.gpsimd.partition_broadcast(bc[:], rsq_i[:])
                xq = tiles[("xq", qb)]
                for e in range(NE):
                    nc.vector.tensor_mul(xq[:, e, :], xq[:, e, :], bc[:])

            def emit_qkv(qb):
                xq = tiles.pop(("xq", qb))
                cols = slice(qb * QB, (qb + 1) * QB)
                q_ph = s12.tile([P, H_LOC, QB], bf16, tag="q_ph", bufs=1, name=f"q{qb}")
                for h in range(H_LOC):
                    wq_sb = s12.tile([P, NE * HEAD_DIM], bf16, tag="wqk", bufs=3)
                    nc.sync.dma_start(wq_sb[:], wq[h])
                    wk_sb = s12.tile([P, NE * HEAD_DIM], bf16, tag="wqk", bufs=3)
                    nc.sync.dma_start(wk_sb[:], wk[h])
                    q_ps = psum.tile([P, QB], f32, tag="pC", bufs=2)
                    for e in range(NE):
                        nc.tensor.matmul(q_ps[:], wq_sb[:, e * P:(e + 1) * P],
                                         xq[:, e, :],
                                         start=(e == 0), stop=(e == NE - 1))
                    nc.vector.tensor_copy(q_ph[:, h, :], q_ps[:])
                    k_ps = psum.tile([P, QB], f32, tag="pC", bufs=2)
                    for e in range(NE):
                        nc.tensor.matmul(k_ps[:], wk_sb[:, e * P:(e + 1) * P],
                                         xq[:, e, :],
                                         start=(e == 0), stop=(e == NE - 1))
                    nc.scalar.activation(k_store[:, h, cols], k_ps[:], AF.Copy)
                # v: e-outer with wv streamed; 4 row-chunk accumulators borrow
                # the pA/pB PSUM slots (idle between attention blocks)
                v_ps = [
                    psum.tile([P, HD], f32, tag=t, bufs=2, name=f"v_ps{i}")
                    for i, t in enumerate(("pA", "pA", "pB", "pB"))
                ]
                for e in range(NE):
                    wv_e = s12.tile([P, HD], bf16, tag="wv_e", bufs=3)
                    nc.sync.dma_start(wv_e[:], wv[:, e * HD:(e + 1) * HD])
                    for rc in range(QB // P):
                        nc.tensor.matmul(v_ps[rc][:], xq[:, e, rc * P:(rc + 1) * P],
                                         wv_e[:],
                                         start=(e == 0), stop=(e == NE - 1))
                for rc in range(QB // P):
                    rcg = qb * (QB // P) + rc
                    nc.vector.tensor_copy(
                        v_store[:, rcg].rearrange("p h d -> p (h d)"), v_ps[rc][:])
                return q_ph

            def emit_attention(qb, q_ph):
                ao_ph = s12.tile([P, H_LOC, QB], bf16, tag="ao_ph", bufs=1, name=f"ao{qb}")
                nk = (qb + 1) * (QB // P)
                for h in range(H_LOC):
                    pv_ps = psum.tile([P, QB], f32, tag="pB", bufs=2)
                    sum_ps = psum.tile([1, QB], f32, tag="acc", bufs=2)
                    lg_tiles = {}

                    def emit_lg(kc):
                        lg = psum.tile([P, QB], f32, tag="pA", bufs=2)
                        nc.tensor.matmul(
                            lg[:], k_store[:, h, kc * P:(kc + 1) * P],
                            q_ph[:, h, :], start=True, stop=True)
                        lg_tiles[kc] = lg

                    emit_lg(0)
                    for kc in range(nk):
                        if kc + 1 < nk:
                            emit_lg(kc + 1)
                        lg = lg_tiles.pop(kc)
                        expt = s12.tile([P, QB], bf16, tag="expt", bufs=2)
                        nc.scalar.activation(expt[:], lg[:], AF.Exp, scale=INV_SQRT_D)
                        j = kc - qb * (QB // P)
                        if j >= 0:
                            off = (3 - j) * P
                            nc.vector.tensor_mul(expt[:], expt[:],
                                                 mask_sb[:, off:off + QB])
                        first, last = kc == 0, kc == nk - 1
                        nc.tensor.matmul(pv_ps[:], v_store[:, kc, h, :], expt[:],
                                         start=first, stop=last)
                        nc.tensor.matmul(sum_ps[:], ones_sb[:], expt[:],
                                         start=first, stop=last)
                    rec = s12.tile([1, QB], f32, tag="rec", bufs=2)
                    nc.vector.reciprocal(rec[:], sum_ps[:])
                    rbc = s12.tile([P, QB], f32, tag="bc", bufs=2)
                    nc.gpsimd.partition_broadcast(rbc[:], rec[:])
                    nc.vector.tensor_mul(ao_ph[:, h, :], pv_ps[:], rbc[:])
                return ao_ph

            def emit_ao_stage(qb, ao_ph):
                nc.sync.dma_start(r3(ag1_in[qb]), ao_ph[:])
                nc.gpsimd.collective_compute(
                    "AllGather", mybir.AluOpType.bypass, replica_groups=GROUPS,
                    ins=[ag1_in[qb][:].opt()], outs=[ag1_out[qb][:].opt()],
                )

            def emit_ago_dma(qb, engine=None):
                ago = big16(f"ago{qb}")
                (engine or nc.sync).dma_start(ago[:], r3(ag1_out[qb]))
                tiles[("ago", qb)] = ago

            def emit_outproj(qb):
                """Full out-projection for my E-shard from gathered heads, then
                x2 = xte + proj; stage x2 (fp32->x2_scr, bf16->ag2_in)."""
                cols = slice(qb * QB, (qb + 1) * QB)
                half, ch = qb // 2, (qb % 2) * QB
                ago = tiles.pop(("ago", qb))
                pr_ps = [
                    psum.tile([P, QB], f32, tag=t, bufs=2, name=f"pr{qb}_{i}")
                    for i, t in enumerate(("pA", "pA", "pB", "pB"))
                ]
                for c in range(NHD):
                    for e4 in range(H_LOC):
                        nc.tensor.matmul(pr_ps[e4][:],
                                         wo_sb[:, c, e4 * P:(e4 + 1) * P],
                                         ago[:, c, :],
                                         start=(c == 0), stop=(c == NHD - 1))
                for e4 in range(H_LOC):
                    xe_c = s12.tile([P, QB], f32, tag="xe", bufs=2)
                    nc.sync.dma_start(xe_c[:], r3(xte, cols)[:, e4, :])
                    x2_c = s12.tile([P, QB], f32, tag="x2", bufs=2)
                    nc.vector.tensor_add(x2_c[:], pr_ps[e4][:], xe_c[:])
                    nc.sync.dma_start(r3(x2_scr, cols)[:, e4, :], x2_c[:])
                    x2b_c = s12.tile([P, QB], bf16, tag="x2b", bufs=2)
                    nc.vector.tensor_copy(x2b_c[:], x2_c[:])
                    nc.sync.dma_start(
                        r3(ag2_in[half], slice(ch, ch + QB))[:, e4, :], x2b_c[:])

            def emit_ag2(half):
                nc.gpsimd.collective_compute(
                    "AllGather", mybir.AluOpType.bypass, replica_groups=GROUPS,
                    ins=[ag2_in[half][:].opt()], outs=[ag2_out[half][:].opt()],
                )

            # ---------- stage 5 (FFN) + stage 6 pieces ----------
            def emit_n2_dma(g):
                half, ch = g // 2, (g % 2) * QB
                n2 = big16(f"n2_{g}")
                nc.sync.dma_start(n2[:], r3(ag2_out[half], slice(ch, ch + QB)))
                tiles[("n2w", g)] = n2

            def emit_rms2(g):
                n2 = tiles.pop(("n2w", g))
                ms2 = psum.tile([1, QB], f32, tag="acc", bufs=2, name=f"ms2_{g}")
                for e in range(NE):
                    sq2 = s12.tile([P, QB], bf16, tag="sq", bufs=2)
                    nc.vector.tensor_mul(sq2[:], n2[:, e, :], n2[:, e, :])
                    nc.tensor.matmul(ms2[:], ones_sb[:], sq2[:],
                                     start=(e == 0), stop=(e == NE - 1))
                rsq2 = s12.tile([1, QB], f32, tag="rsq", bufs=1)
                nc.scalar.activation(rsq2[:], ms2[:], AF.Sqrt, bias=eps_sb[:],
                                     scale=1.0 / EMBED)
                rsq2_i = s12.tile([1, QB], f32, tag="rsqi", bufs=1)
                nc.vector.reciprocal(rsq2_i[:], rsq2[:])
                bc2 = s12.tile([P, QB], f32, tag="bc", bufs=2)
                nc.gpsimd.partition_broadcast(bc2[:], rsq2_i[:])
                n8 = s12.tile([P, NE, QB], f8, tag="n8", bufs=2, name=f"n8_{g}")
                for e in range(NE):
                    nc.vector.tensor_mul(n8[:, e, :], n2[:, e, :], bc2[:])
                tiles[("n8", g)] = n8

            def dr2(w_sb, e2):
                """[P, 256] fp8 slice -> [P, 2, 128] DoubleRow stationary view."""
                return w_sb[:, 2 * e2 * P:(2 * e2 + 2) * P].rearrange(
                    "p (two j) -> p two j", two=2)

            def emit_gateup(g):
                n8 = tiles[("n8", g)]
                act = s12.tile([P, NF, QB], f8, tag="act", bufs=2, name=f"act{g}")
                for f in range(NF):
                    wg_sb = s12.tile([P, NE * P], f8, tag="wgu", bufs=3)
                    nc.sync.dma_start(wg_sb[:], wg[f])
                    wu_sb = s12.tile([P, NE * P], f8, tag="wgu", bufs=3)
                    nc.sync.dma_start(wu_sb[:], wu[f])
                    g_ps = psum.tile([P, QB], f32, tag="pA", bufs=2)
                    for e2 in range(NE // 2):
                        nc.tensor.matmul(g_ps[:], dr2(wg_sb, e2),
                                         n8[:, 2 * e2:2 * e2 + 2, :],
                                         start=(e2 == 0), stop=(e2 == NE // 2 - 1),
                                         perf_mode=DR)
                    u_ps = psum.tile([P, QB], f32, tag="pB", bufs=2)
                    for e2 in range(NE // 2):
                        nc.tensor.matmul(u_ps[:], dr2(wu_sb, e2),
                                         n8[:, 2 * e2:2 * e2 + 2, :],
                                         start=(e2 == 0), stop=(e2 == NE // 2 - 1),
                                         perf_mode=DR)
                    gel = s12.tile([P, QB], bf16, tag="gel", bufs=2)
                    nc.scalar.activation(gel[:], g_ps[:], AF.Gelu_apprx_tanh,
                                         scale=1.0 / WG_SCALE)
                    nc.vector.tensor_mul(act[:, f, :], gel[:], u_ps[:])
                tiles[("act", g)] = act

            def emit_downs(g):
                act = tiles.pop(("act", g))
                tiles.pop(("n8", g), None)
                for e in range(NE):
                    wd_sb = s12.tile([P, NF * P], f8, tag="wd", bufs=2)
                    nc.sync.dma_start(wd_sb[:], wd[e])
                    d_ps = psum.tile([P, QB], f32, tag="pC", bufs=2)
                    for f2 in range(NF // 2):
                        nc.tensor.matmul(d_ps[:], dr2(wd_sb, f2),
                                         act[:, 2 * f2:2 * f2 + 2, :],
                                         start=(f2 == 0), stop=(f2 == NF // 2 - 1),
                                         perf_mode=DR)
                    d_sb = s12.tile([P, QB], bf16, tag="dstage", bufs=2)
                    nc.scalar.activation(d_sb[:], d_ps[:], AF.Copy,
                                         scale=1.0 / (WU_SCALE * WD_SCALE))
                    nc.sync.dma_start(r3(rs2_in[g][e * P:(e + 1) * P, :]), d_sb[:])
                nc.gpsimd.collective_compute(
                    "ReduceScatter", mybir.AluOpType.add, replica_groups=GROUPS,
                    ins=[rs2_in[g][:].opt()], outs=[rs2_out[g][:].opt()],
                )

            def emit_stage6(g):
                cols = slice(g * QB, (g + 1) * QB)
                for e4 in range(H_LOC):
                    fsum = s12.tile([P, QB], bf16, tag="fsum", bufs=1)
                    nc.sync.dma_start(fsum[:], r3(rs2_out[g])[:, e4, :])
                    x2r = s12.tile([P, QB], f32, tag="x2r", bufs=1)
                    nc.sync.dma_start(x2r[:], r3(x2_scr, cols)[:, e4, :])
                    fin = s12.tile([P, QB], f32, tag="fin", bufs=1)
                    nc.vector.tensor_add(fin[:], fsum[:], x2r[:])
                    nc.sync.dma_start(r3(out_ext, cols)[:, e4, :], fin[:])

            # ---------- schedule ----------
            # outproj(qb-1) is emitted AFTER attention(qb): the ago load's
            # AG1-completion wait sits on the (idle) sync queue during
            # attention, and the PE reaches outproj long after AG1 finished.
            emit_xb_dma(0)
            emit_sq_ms(0)
            emit_norm_tail(0)
            for qb in range(NQB):
                q_ph = emit_qkv(qb)
                if qb + 1 < NQB:
                    emit_xb_dma(qb + 1)
                if qb == 0:
                    nc.sync.dma_start(wo_sb[:].rearrange("p a b -> p (a b)"), wout2[:])
                if qb >= 1:
                    emit_ago_dma(qb - 1)
                if qb == 3:
                    emit_n2_dma(0)  # loads during attention(3); AG2(0) done by then
                ao_ph = emit_attention(qb, q_ph)
                emit_ao_stage(qb, ao_ph)
                if qb >= 1:
                    emit_outproj(qb - 1)
                if qb == 2:
                    emit_ag2(0)
                if qb + 1 < NQB:
                    emit_sq_ms(qb + 1)
                    emit_norm_tail(qb + 1)

            # ago(3) load issues on the scalar queue at FFN start: its AG1(3)
            # wait only delays gelu issuance (not PE), and outproj(3) runs
            # after downs(0) when the load is long done.
            emit_ago_dma(3, engine=nc.scalar)
            emit_n2_dma(1)
            emit_rms2(0)
            emit_gateup(0)
            emit_downs(0)
            emit_outproj(3)
            emit_ag2(1)
            emit_rms2(1)
            emit_gateup(1)
            emit_downs(1)
            emit_n2_dma(2)
            emit_rms2(2)
            emit_gateup(2)
            emit_stage6(0)
            emit_downs(2)
            emit_n2_dma(3)
            emit_rms2(3)
            emit_gateup(3)
            emit_stage6(1)
            emit_downs(3)
            emit_stage6(2)
            emit_stage6(3)


# ============================ host side ============================


def _prep_core_inputs(inputs):
    """Shard + transpose + fold rms scales into weights; pre-arrange streamed
    weights into per-partition-contiguous [128, X] layouts. 8 in_maps."""
    import ml_dtypes

    bf16 = ml_dtypes.bfloat16

    x = np.asarray(inputs["x"], np.float32)          # [B, S, E]
    w_qkv = np.asarray(inputs["w_qkv"], np.float32)  # [E, H, 3D]
    w_out = np.asarray(inputs["w_out"], np.float32)  # [H, D, E]
    w_gate = np.asarray(inputs["w_gate"], np.float32)
    w_up = np.asarray(inputs["w_up"], np.float32)
    w_down = np.asarray(inputs["w_down"], np.float32)
    scale1 = np.asarray(inputs["scale1"], np.float32)
    scale2 = np.asarray(inputs["scale2"], np.float32)

    wqkv_s = w_qkv * scale1[:, None, None]
    wq_f = wqkv_s[:, :, 0:HEAD_DIM]
    wk_f = wqkv_s[:, :, HEAD_DIM:2 * HEAD_DIM]
    wv_f = wqkv_s[:, :, 2 * HEAD_DIM:3 * HEAD_DIM]
    wout_f = w_out.reshape(HEADS * HEAD_DIM, EMBED)
    wg_s = w_gate * scale2[:, None]
    wu_s = w_up * scale2[:, None]

    kp = np.arange(P)[:, None]
    m = np.arange(QB + 3 * P)[None, :]
    masks = (m >= kp + 3 * P).astype(bf16)  # mask_j = masks[:, (3-j)*128 : (3-j)*128+512]

    def prep_qk(w):  # [E, H_LOC, D] -> [H_LOC, P, NE*D]: [h,p,e*D+d] = w[e*128+p,h,d]
        return np.ascontiguousarray(
            w.reshape(NE, P, H_LOC, HEAD_DIM).transpose(2, 1, 0, 3)
            .reshape(H_LOC, P, NE * HEAD_DIM).astype(bf16))

    def prep_colmajor(w, nchunk):  # [K, M] -> [P, nchunk*M]: [p, c*M+m] = w[c*128+p, m]
        k, mm = w.shape
        return np.ascontiguousarray(
            w.reshape(nchunk, P, mm).transpose(1, 0, 2).reshape(P, nchunk * mm)
            .astype(bf16))

    f8np = ml_dtypes.float8_e4m3

    def prep_fchunk(w, nout, scale):  # [K, F] -> [F/128, P, (K/128)*128], fp8 x scale
        k, ff = w.shape
        nk = k // P
        ws = np.clip(w * scale, -240.0, 240.0)
        return np.ascontiguousarray(
            ws.reshape(nk, P, nout, P).transpose(2, 1, 0, 3)
            .reshape(nout, P, nk * P).astype(f8np))

    in_maps = []
    for c in range(N_CORES):
        b, t = divmod(c, TP)
        hs = slice(H_LOC * t, H_LOC * (t + 1))
        fs = slice(F_LOC * t, F_LOC * (t + 1))
        es = slice(ROWS_T * t, ROWS_T * (t + 1))
        xtb = np.ascontiguousarray(x[b].T)  # [E, S]
        in_maps.append(
            {
                "xb": np.ascontiguousarray(xtb.astype(bf16)),
                "xte": np.ascontiguousarray(xtb[es, :]),
                "wq": prep_qk(wq_f[:, hs, :]),
                "wk": prep_qk(wk_f[:, hs, :]),
                "wv": prep_colmajor(wv_f[:, hs, :].reshape(EMBED, HD), NE),
                "wout2": prep_colmajor(wout_f[:, es], NHD),
                "wg": prep_fchunk(wg_s[:, fs], NF, WG_SCALE),
                "wu": prep_fchunk(wu_s[:, fs], NF, WU_SCALE),
                "wd": prep_fchunk(w_down[fs, :], NE, WD_SCALE),
                "masks": np.ascontiguousarray(masks),
                "ones": np.ones((P, 1), bf16),
            }
        )
    return in_maps


def _install_profile_hook():
    import sys
    import types

    try:
        import antenv.axon_hooks  # noqa: F401

        return
    except ImportError:
        pass
    try:
        from trn_agent_boot.trn_boot import _ntff_profile_via_ctypes

        _hook = _ntff_profile_via_ctypes("/opt/axon/libaxon_pjrt.so")
        _mod = types.ModuleType("antenv.axon_hooks")
        _mod.get_axon_ntff_profile_hook = lambda: _hook
        sys.modules["antenv.axon_hooks"] = _mod
    except Exception:
        pass


def _run(nc, in_maps, trace=False, trace_cores=None):
    _install_profile_hook()
    from concourse.bass_utils import run_bass_kernel_spmd

    return run_bass_kernel_spmd(
        nc,
        in_maps,
        core_ids=list(range(N_CORES)),
        trace=trace,
        trace_cores=trace_cores,
    )


def kernel(**inputs):
    if "nc" not in _NC_CACHE:
        _NC_CACHE["nc"] = build_kernel()
    nc = _NC_CACHE["nc"]
    in_maps = _prep_core_inputs(inputs)
    res = _run(nc, in_maps)
    out = np.empty((BATCH, SEQ, EMBED), np.float32)
    for c in range(N_CORES):
        b, t = divmod(c, TP)
        out[b, :, ROWS_T * t:ROWS_T * (t + 1)] = res.results[c]["out"].T
    return out


if __name__ == "__main__":
    build_kernel()
    print("build ok")


# revision 19
# speedup vs baseline: 1.0782x; 1.0782x over previous
"""Trainium2 Bass kernel for a dense transformer block (RMSNorm->MHA->res, RMSNorm->SwiGLU-FFN->res).

Sharding over 8 NeuronCores: fsdp=2 (batch) x tp=4 (attention heads / FFN hidden).
Core 4*b + t handles batch b with TP-rank t (heads 4t..4t+3, FFN hidden cols 2048t..2048(t+1)).

All matmul operands are bf16 (full PE rate, FWL weight loads, half the SBUF/DMA
bytes); accumulation and the residual stream stay fp32.  On-device activations are
feature-major ([features, rows]) so matmuls chain without transposes.  Streamed
weights are pre-arranged on the host into per-partition-contiguous [128, X]
layouts so every weight DMA is a single >=2KB descriptor per partition.

Collective plan (per 4-core TP group, all payloads bf16 -- CCE adds in bf16):
  AG1(qb): AllGather of per-rank attention-head outputs ao [512(hd),512] for
           q-block qb -> [2048(hd),512].  Each rank then computes the FULL
           out-projection for its 512-row E-shard locally (same FLOPs as the
           head-sharded projection; an AllGather costs half a ReduceScatter on
           the wire and removes partial-sum staging).
  AG2(h):  AllGather of raw x2 E-shards [512,1024] -> [2048,1024] per row-half.
           rms2 runs locally AFTER the gather (no mean-square AllReduce).
  RS2(g):  ReduceScatter of FFN down partials [2048,512] -> [512,512] per
           512-row group.
Emission order gives every collective >=1 compute phase of slack before its
first consumer's DMA is issued, so engine sequencers never block on collective
semaphores.  Final output per core: [its 512 E-features, 2048 rows].
"""

import numpy as np

EMBED = 2048
HEADS = 16
HEAD_DIM = 128
FF_HID = 8192
BATCH = 2
SEQ = 2048
EPS = 1e-6

N_CORES = 8
TP = 4
GROUPS = [[0, 1, 2, 3], [4, 5, 6, 7]]
H_LOC = HEADS // TP          # 4 heads per core
F_LOC = FF_HID // TP         # 2048 ffn-hidden per core
ROWS = SEQ                   # 2048 rows per batch
ROWS_T = ROWS // TP          # 512 rows per tp-rank (E-shard size)
P = 128
NE = EMBED // P              # 16 embed chunks
NF = F_LOC // P              # 16 ffn chunks
NR = ROWS // P               # 16 row chunks
QB = 512                     # q-block / phase row count / matmul moving size
NQB = ROWS // QB             # 4 phases
RH = 1024                    # ffn row-half (AG2 granularity)
HD = H_LOC * HEAD_DIM        # 512 local head-dim rows
NHD = HEADS * HEAD_DIM // P  # 16 gathered hd chunks
INV_SQRT_D = float(1.0 / np.sqrt(HEAD_DIM))
# fp8(e4m3, max 240) FFN weight pre-scales keep the tiny (std~0.01) weights out
# of the subnormal range; the gelu input scale and the down-stage copy undo them.
WG_SCALE = 64.0
WU_SCALE = 16.0
WD_SCALE = 64.0

_NC_CACHE = {}


def build_kernel():
    import concourse.mybir as mybir
    import concourse.tile as tile
    from concourse import bacc

    f32 = mybir.dt.float32
    bf16 = mybir.dt.bfloat16

    nc = bacc.Bacc("TRN2", target_bir_lowering=False, debug=False, num_devices=N_CORES)

    io = {}
    # xb is host-pre-arranged [128, qb*8192 + e*512 + q] so each phase load is
    # one 16KB-contiguous descriptor per partition
    io["xb"] = nc.dram_tensor("xb", [P, NQB * NE * QB], bf16, kind="ExternalInput").ap()
    io["xte"] = nc.dram_tensor("xte", [ROWS_T, ROWS], f32, kind="ExternalInput").ap()
    io["wq"] = nc.dram_tensor("wq", [H_LOC, P, NE * HEAD_DIM], bf16, kind="ExternalInput").ap()
    io["wk"] = nc.dram_tensor("wk", [H_LOC, P, NE * HEAD_DIM], bf16, kind="ExternalInput").ap()
    io["wv"] = nc.dram_tensor("wv", [P, NE * HD], bf16, kind="ExternalInput").ap()
    io["wout2"] = nc.dram_tensor("wout2", [P, NHD * ROWS_T], bf16, kind="ExternalInput").ap()
    f8 = mybir.dt.float8e4
    io["wg"] = nc.dram_tensor("wg", [NF, P, NE * P], f8, kind="ExternalInput").ap()
    io["wu"] = nc.dram_tensor("wu", [NF, P, NE * P], f8, kind="ExternalInput").ap()
    io["wd"] = nc.dram_tensor("wd", [NE, P, NF * P], f8, kind="ExternalInput").ap()
    io["masks"] = nc.dram_tensor("masks", [P, QB + 3 * P], bf16, kind="ExternalInput").ap()
    io["ones"] = nc.dram_tensor("ones", [P, 1], bf16, kind="ExternalInput").ap()
    io["out"] = nc.dram_tensor("out", [ROWS_T, ROWS], f32, kind="ExternalOutput").ap()

    with tile.TileContext(nc) as tc:
        _emit(tc, nc, io)
    nc.compile()
    return nc


def _emit(tc, nc, io):
    from contextlib import ExitStack

    import concourse.mybir as mybir

    f32 = mybir.dt.float32
    bf16 = mybir.dt.bfloat16
    f8 = mybir.dt.float8e4
    DR = mybir.MatmulPerfMode.DoubleRow
    AF = mybir.ActivationFunctionType

    xb_in, xte, wq, wk, wv = io["xb"], io["xte"], io["wq"], io["wk"], io["wv"]
    ones_in = io["ones"]
    wout2, wg, wu, wd, masks = io["wout2"], io["wg"], io["wu"], io["wd"], io["masks"]
    out_ext = io["out"]

    def r3(ap2d, cols=None):
        """[(o p), q] dram view -> [p, o, q]; optionally slice columns first."""
        v = ap2d if cols is None else ap2d[:, cols]
        return v.rearrange("(o p) q -> p o q", p=P)

    ctx = ExitStack()
    with ctx:
        consts = ctx.enter_context(tc.tile_pool(name="consts", bufs=1))
        dram = ctx.enter_context(tc.tile_pool(name="dram", bufs=1, space="DRAM"))
        psum = ctx.enter_context(tc.tile_pool(name="psum", bufs=2, space="PSUM"))

        ones_sb = consts.tile([P, 1], bf16)
        nc.sync.dma_start(ones_sb[:], ones_in[:])
        eps_sb = consts.tile([1, 1], f32)
        nc.vector.memset(eps_sb[:], EPS)
        mask_sb = consts.tile([P, QB + 3 * P], bf16)
        nc.sync.dma_start(mask_sb[:], masks[:])
        wo_sb = consts.tile([P, NHD, ROWS_T], bf16)  # DMA deferred off the startup path

        # one dram tile per collective slice — a shared tensor would add false
        # whole-tensor deps (e.g. an ag2_out[0] reader waiting on AG2(1)'s write)
        ag1_in = [dram.tile([HD, QB], bf16, name=f"ag1i{i}") for i in range(NQB)]
        ag1_out = [dram.tile([HEADS * HEAD_DIM, QB], bf16, name=f"ag1o{i}") for i in range(NQB)]
        ag2_in = [dram.tile([ROWS_T, QB], bf16, name=f"ag2i{i}") for i in range(NQB)]
        ag2_out = [dram.tile([EMBED, QB], bf16, name=f"ag2o{i}") for i in range(NQB)]
        rs2_in = [dram.tile([EMBED, QB], bf16, name=f"rs2i{i}") for i in range(NQB)]
        rs2_out = [dram.tile([ROWS_T, QB], bf16, name=f"rs2o{i}") for i in range(NQB)]
        x2_scr = dram.tile([ROWS_T, ROWS], f32)

        with tc.tile_pool(name="s12", bufs=1) as s12:
            k_store = s12.tile([P, H_LOC, ROWS], bf16)
            v_store = s12.tile([P, NR, H_LOC, HEAD_DIM], bf16)

            tiles = {}

            def big16(name):
                """3-slot rotation shared by xq(0..3), ago(0..3) and n2(0..3) —
                16KB/partition tiles whose live ranges interleave exactly 3-deep."""
                return s12.tile([P, NE, QB], bf16, tag="big16", bufs=3, name=name)

            # ---------- stage 1+2 pieces ----------
            def emit_xb_dma(qb):
                xq = big16(f"xq{qb}")
                nc.sync.dma_start(xq[:].rearrange("p a b -> p (a b)"),
                                  xb_in[:, qb * NE * QB:(qb + 1) * NE * QB])
                tiles[("xq", qb)] = xq

            def emit_sq_ms(qb):
                ms = psum.tile([1, QB], f32, tag="acc", bufs=2, name=f"ms{qb}")
                xq = tiles[("xq", qb)]
                for e in range(NE):
                    sq = s12.tile([P, QB], bf16, tag="sq", bufs=2)
                    nc.vector.tensor_mul(sq[:], xq[:, e, :], xq[:, e, :])
                    nc.tensor.matmul(ms[:], ones_sb[:], sq[:],
                                     start=(e == 0), stop=(e == NE - 1))
                tiles[("ms", qb)] = ms

            def emit_norm_tail(qb):
                ms = tiles.pop(("ms", qb))
                rsq = s12.tile([1, QB], f32, tag="rsq", bufs=1)
                nc.scalar.activation(rsq[:], ms[:], AF.Sqrt, bias=eps_sb[:], scale=1.0 / EMBED)
                rsq_i = s12.tile([1, QB], f32, tag="rsqi", bufs=1)
                nc.vector.reciprocal(rsq_i[:], rsq[:])
                bc = s12.tile([P, QB], f32, tag="bc", bufs=2)
                nc.gpsimd.partition_broadcast(bc[:], rsq_i[:])
                xq = tiles[("xq", qb)]
                for e in range(NE):
                    nc.vector.tensor_mul(xq[:, e, :], xq[:, e, :], bc[:])

            def emit_qkv(qb):
                xq = tiles.pop(("xq", qb))
                cols = slice(qb * QB, (qb + 1) * QB)
                q_ph = s12.tile([P, H_LOC, QB], bf16, tag="q_ph", bufs=1, name=f"q{qb}")
                for h in range(H_LOC):
                    wq_sb = s12.tile([P, NE * HEAD_DIM], bf16, tag="wqk", bufs=3)
                    nc.sync.dma_start(wq_sb[:], wq[h])
                    wk_sb = s12.tile([P, NE * HEAD_DIM], bf16, tag="wqk", bufs=3)
                    nc.sync.dma_start(wk_sb[:], wk[h])
                    q_ps = psum.tile([P, QB], f32, tag="pC", bufs=2)
                    for e in range(NE):
                        nc.tensor.matmul(q_ps[:], wq_sb[:, e * P:(e + 1) * P],
                                         xq[:, e, :],
                                         start=(e == 0), stop=(e == NE - 1))
                    nc.vector.tensor_copy(q_ph[:, h, :], q_ps[:])
                    k_ps = psum.tile([P, QB], f32, tag="pC", bufs=2)
                    for e in range(NE):
                        nc.tensor.matmul(k_ps[:], wk_sb[:, e * P:(e + 1) * P],
                                         xq[:, e, :],
                                         start=(e == 0), stop=(e == NE - 1))
                    nc.scalar.activation(k_store[:, h, cols], k_ps[:], AF.Copy)
                # v: e-outer with wv streamed; 4 row-chunk accumulators borrow
                # the pA/pB PSUM slots (idle between attention blocks)
                v_ps = [
                    psum.tile([P, HD], f32, tag=t, bufs=2, name=f"v_ps{i}")
                    for i, t in enumerate(("pA", "pA", "pB", "pB"))
                ]
                for e in range(NE):
                    wv_e = s12.tile([P, HD], bf16, tag="wv_e", bufs=3)
                    nc.sync.dma_start(wv_e[:], wv[:, e * HD:(e + 1) * HD])
                    for rc in range(QB // P):
                        nc.tensor.matmul(v_ps[rc][:], xq[:, e, rc * P:(rc + 1) * P],
                                         wv_e[:],
                                         start=(e == 0), stop=(e == NE - 1))
                for rc in range(QB // P):
                    rcg = qb * (QB // P) + rc
                    nc.vector.tensor_copy(
                        v_store[:, rcg].rearrange("p h d -> p (h d)"), v_ps[rc][:])
                return q_ph

            def emit_attention(qb, q_ph):
                ao_ph = s12.tile([P, H_LOC, QB], bf16, tag="ao_ph", bufs=1, name=f"ao{qb}")
                nk = (qb + 1) * (QB // P)
                for h in range(H_LOC):
                    pv_ps = psum.tile([P, QB], f32, tag="pB", bufs=2)
                    sum_ps = psum.tile([1, QB], f32, tag="acc", bufs=2)
                    lg_tiles = {}

                    def emit_lg(kc):
                        lg = psum.tile([P, QB], f32, tag="pA", bufs=2)
                        nc.tensor.matmul(
                            lg[:], k_store[:, h, kc * P:(kc + 1) * P],
                            q_ph[:, h, :], start=True, stop=True)
                        lg_tiles[kc] = lg

                    emit_lg(0)
                    for kc in range(nk):
                        if kc + 1 < nk:
                            emit_lg(kc + 1)
                        lg = lg_tiles.pop(kc)
                        expt = s12.tile([P, QB], bf16, tag="expt", bufs=2)
                        nc.scalar.activation(expt[:], lg[:], AF.Exp, scale=INV_SQRT_D)
                        j = kc - qb * (QB // P)
                        if j >= 0:
                            off = (3 - j) * P
                            nc.vector.tensor_mul(expt[:], expt[:],
                                                 mask_sb[:, off:off + QB])
                        first, last = kc == 0, kc == nk - 1
                        nc.tensor.matmul(pv_ps[:], v_store[:, kc, h, :], expt[:],
                                         start=first, stop=last)
                        nc.tensor.matmul(sum_ps[:], ones_sb[:], expt[:],
                                         start=first, stop=last)
                    rec = s12.tile([1, QB], f32, tag="rec", bufs=2)
                    nc.vector.reciprocal(rec[:], sum_ps[:])
                    rbc = s12.tile([P, QB], f32, tag="bc", bufs=2)
                    nc.gpsimd.partition_broadcast(rbc[:], rec[:])
                    nc.vector.tensor_mul(ao_ph[:, h, :], pv_ps[:], rbc[:])
                return ao_ph

            def emit_ao_stage(qb, ao_ph):
                nc.sync.dma_start(r3(ag1_in[qb]), ao_ph[:])
                nc.gpsimd.collective_compute(
                    "AllGather", mybir.AluOpType.bypass, replica_groups=GROUPS,
                    ins=[ag1_in[qb][:].opt()], outs=[ag1_out[qb][:].opt()],
                )

            def emit_ago_dma(qb, engine=None):
                ago = big16(f"ago{qb}")
                (engine or nc.sync).dma_start(ago[:], r3(ag1_out[qb]))
                tiles[("ago", qb)] = ago

            def emit_outproj(qb):
                """Full out-projection for my E-shard from gathered heads, then
                x2 = xte + proj; stage x2 (fp32->x2_scr, bf16->ag2_in)."""
                cols = slice(qb * QB, (qb + 1) * QB)
                ago = tiles.pop(("ago", qb))
                pr_ps = [
                    psum.tile([P, QB], f32, tag=t, bufs=2, name=f"pr{qb}_{i}")
                    for i, t in enumerate(("pA", "pA", "pB", "pB"))
                ]
                for c in range(NHD):
                    for e4 in range(H_LOC):
                        nc.tensor.matmul(pr_ps[e4][:],
                                         wo_sb[:, c, e4 * P:(e4 + 1) * P],
                                         ago[:, c, :],
                                         start=(c == 0), stop=(c == NHD - 1))
                for e4 in range(H_LOC):
                    xe_c = s12.tile([P, QB], f32, tag="xe", bufs=2)
                    nc.sync.dma_start(xe_c[:], r3(xte, cols)[:, e4, :])
                    x2_c = s12.tile([P, QB], f32, tag="x2", bufs=2)
                    nc.vector.tensor_add(x2_c[:], pr_ps[e4][:], xe_c[:])
                    nc.sync.dma_start(r3(x2_scr, cols)[:, e4, :], x2_c[:])
                    x2b_c = s12.tile([P, QB], bf16, tag="x2b", bufs=2)
                    nc.vector.tensor_copy(x2b_c[:], x2_c[:])
                    nc.sync.dma_start(r3(ag2_in[qb])[:, e4, :], x2b_c[:])

            def emit_ag2(qb):
                nc.gpsimd.collective_compute(
                    "AllGather", mybir.AluOpType.bypass, replica_groups=GROUPS,
                    ins=[ag2_in[qb][:].opt()], outs=[ag2_out[qb][:].opt()],
                )

            # ---------- stage 5 (FFN) + stage 6 pieces ----------
            def emit_n2_dma(g):
                n2 = big16(f"n2_{g}")
                nc.sync.dma_start(n2[:], r3(ag2_out[g]))
                tiles[("n2w", g)] = n2

            def emit_rms2(g):
                n2 = tiles.pop(("n2w", g))
                ms2 = psum.tile([1, QB], f32, tag="acc", bufs=2, name=f"ms2_{g}")
                for e in range(NE):
                    sq2 = s12.tile([P, QB], bf16, tag="sq", bufs=2)
                    nc.vector.tensor_mul(sq2[:], n2[:, e, :], n2[:, e, :])
                    nc.tensor.matmul(ms2[:], ones_sb[:], sq2[:],
                                     start=(e == 0), stop=(e == NE - 1))
                rsq2 = s12.tile([1, QB], f32, tag="rsq", bufs=1)
                nc.scalar.activation(rsq2[:], ms2[:], AF.Sqrt, bias=eps_sb[:],
                                     scale=1.0 / EMBED)
                rsq2_i = s12.tile([1, QB], f32, tag="rsqi", bufs=1)
                nc.vector.reciprocal(rsq2_i[:], rsq2[:])
                bc2 = s12.tile([P, QB], f32, tag="bc", bufs=2)
                nc.gpsimd.partition_broadcast(bc2[:], rsq2_i[:])
                n8 = s12.tile([P, NE, QB], f8, tag="n8", bufs=2, name=f"n8_{g}")
                for e in range(NE):
                    nc.vector.tensor_mul(n8[:, e, :], n2[:, e, :], bc2[:])
                tiles[("n8", g)] = n8

            def dr2(w_sb, e2):
                """[P, 256] fp8 slice -> [P, 2, 128] DoubleRow stationary view."""
                return w_sb[:, 2 * e2 * P:(2 * e2 + 2) * P].rearrange(
                    "p (two j) -> p two j", two=2)

            def emit_gateup(g):
                n8 = tiles[("n8", g)]
                act = s12.tile([P, NF, QB], f8, tag="act", bufs=2, name=f"act{g}")
                for f in range(NF):
                    wg_sb = s12.tile([P, NE * P], f8, tag="wgu", bufs=3)
                    nc.sync.dma_start(wg_sb[:], wg[f])
                    wu_sb = s12.tile([P, NE * P], f8, tag="wgu", bufs=3)
                    nc.sync.dma_start(wu_sb[:], wu[f])
                    g_ps = psum.tile([P, QB], f32, tag="pA", bufs=2)
                    for e2 in range(NE // 2):
                        nc.tensor.matmul(g_ps[:], dr2(wg_sb, e2),
                                         n8[:, 2 * e2:2 * e2 + 2, :],
                                         start=(e2 == 0), stop=(e2 == NE // 2 - 1),
                                         perf_mode=DR)
                    u_ps = psum.tile([P, QB], f32, tag="pB", bufs=2)
                    for e2 in range(NE // 2):
                        nc.tensor.matmul(u_ps[:], dr2(wu_sb, e2),
                                         n8[:, 2 * e2:2 * e2 + 2, :],
                                         start=(e2 == 0), stop=(e2 == NE // 2 - 1),
                                         perf_mode=DR)
                    gel = s12.tile([P, QB], bf16, tag="gel", bufs=2)
                    nc.scalar.activation(gel[:], g_ps[:], AF.Gelu_apprx_tanh,
                                         scale=1.0 / WG_SCALE)
                    nc.vector.tensor_mul(act[:, f, :], gel[:], u_ps[:])
                tiles[("act", g)] = act

            def emit_downs(g):
                act = tiles.pop(("act", g))
                tiles.pop(("n8", g), None)
                for e in range(NE):
                    wd_sb = s12.tile([P, NF * P], f8, tag="wd", bufs=2)
                    nc.sync.dma_start(wd_sb[:], wd[e])
                    d_ps = psum.tile([P, QB], f32, tag="pC", bufs=2)
                    for f2 in range(NF // 2):
                        nc.tensor.matmul(d_ps[:], dr2(wd_sb, f2),
                                         act[:, 2 * f2:2 * f2 + 2, :],
                                         start=(f2 == 0), stop=(f2 == NF // 2 - 1),
                                         perf_mode=DR)
                    d_sb = s12.tile([P, QB], bf16, tag="dstage", bufs=2)
                    nc.scalar.activation(d_sb[:], d_ps[:], AF.Copy,
                                         scale=1.0 / (WU_SCALE * WD_SCALE))
                    nc.sync.dma_start(r3(rs2_in[g][e * P:(e + 1) * P, :]), d_sb[:])
                nc.gpsimd.collective_compute(
                    "ReduceScatter", mybir.AluOpType.add, replica_groups=GROUPS,
                    ins=[rs2_in[g][:].opt()], outs=[rs2_out[g][:].opt()],
                )

            def emit_stage6(g):
                cols = slice(g * QB, (g + 1) * QB)
                for e4 in range(H_LOC):
                    fsum = s12.tile([P, QB], bf16, tag="fsum", bufs=1)
                    nc.sync.dma_start(fsum[:], r3(rs2_out[g])[:, e4, :])
                    x2r = s12.tile([P, QB], f32, tag="x2r", bufs=1)
                    nc.sync.dma_start(x2r[:], r3(x2_scr, cols)[:, e4, :])
                    fin = s12.tile([P, QB], f32, tag="fin", bufs=1)
                    nc.vector.tensor_add(fin[:], fsum[:], x2r[:])
                    nc.sync.dma_start(r3(out_ext, cols)[:, e4, :], fin[:])

            # ---------- schedule ----------
            # outproj(qb-1) is emitted AFTER attention(qb): the ago load's
            # AG1-completion wait sits on the (idle) sync queue during
            # attention, and the PE reaches outproj long after AG1 finished.
            emit_xb_dma(0)
            emit_sq_ms(0)
            emit_norm_tail(0)
            # outproj(qb-2) runs two phases after its AG1: every gather and
            # every gather-output load has >=1 full compute phase of slack.
            # AG2 fires per phase right after its outproj.
            for qb in range(NQB):
                q_ph = emit_qkv(qb)
                if qb + 1 < NQB:
                    emit_xb_dma(qb + 1)
                if qb == 1:
                    nc.sync.dma_start(wo_sb[:].rearrange("p a b -> p (a b)"), wout2[:])
                if qb >= 2:
                    emit_ago_dma(qb - 2)
                if qb == 3:
                    emit_ago_dma(2)
                    emit_n2_dma(0)  # loads during attention(3); AG2(0) done by then
                ao_ph = emit_attention(qb, q_ph)
                emit_ao_stage(qb, ao_ph)
                if qb >= 2:
                    emit_outproj(qb - 2)
                    emit_ag2(qb - 2)
                if qb + 1 < NQB:
                    emit_sq_ms(qb + 1)
                    emit_norm_tail(qb + 1)

            emit_outproj(2)
            emit_ag2(2)
            emit_rms2(0)
            emit_gateup(0)
            # ago(3) load issues on the scalar queue: its AG1(3) wait only
            # delays gelu issuance (not PE), and outproj(3) runs after
            # downs(0) when the load is long done.
            emit_ago_dma(3, engine=nc.scalar)
            emit_n2_dma(1)
            emit_downs(0)
            emit_outproj(3)
            emit_ag2(3)
            emit_rms2(1)
            emit_gateup(1)
            emit_downs(1)
            emit_n2_dma(2)
            emit_rms2(2)
            emit_gateup(2)
            emit_stage6(0)
            emit_downs(2)
            emit_n2_dma(3)
            emit_rms2(3)
            emit_gateup(3)
            emit_stage6(1)
            emit_downs(3)
            emit_stage6(2)
            emit_stage6(3)


# ============================ host side ============================


def _prep_core_inputs(inputs):
    """Shard + transpose + fold rms scales into weights; pre-arrange streamed
    weights into per-partition-contiguous [128, X] layouts. 8 in_maps."""
    import ml_dtypes

    bf16 = ml_dtypes.bfloat16

    x = np.asarray(inputs["x"], np.float32)          # [B, S, E]
    w_qkv = np.asarray(inputs["w_qkv"], np.float32)  # [E, H, 3D]
    w_out = np.asarray(inputs["w_out"], np.float32)  # [H, D, E]
    w_gate = np.asarray(inputs["w_gate"], np.float32)
    w_up = np.asarray(inputs["w_up"], np.float32)
    w_down = np.asarray(inputs["w_down"], np.float32)
    scale1 = np.asarray(inputs["scale1"], np.float32)
    scale2 = np.asarray(inputs["scale2"], np.float32)

    wqkv_s = w_qkv * scale1[:, None, None]
    wq_f = wqkv_s[:, :, 0:HEAD_DIM]
    wk_f = wqkv_s[:, :, HEAD_DIM:2 * HEAD_DIM]
    wv_f = wqkv_s[:, :, 2 * HEAD_DIM:3 * HEAD_DIM]
    wout_f = w_out.reshape(HEADS * HEAD_DIM, EMBED)
    wg_s = w_gate * scale2[:, None]
    wu_s = w_up * scale2[:, None]

    kp = np.arange(P)[:, None]
    m = np.arange(QB + 3 * P)[None, :]
    masks = (m >= kp + 3 * P).astype(bf16)  # mask_j = masks[:, (3-j)*128 : (3-j)*128+512]

    def prep_qk(w):  # [E, H_LOC, D] -> [H_LOC, P, NE*D]: [h,p,e*D+d] = w[e*128+p,h,d]
        return np.ascontiguousarray(
            w.reshape(NE, P, H_LOC, HEAD_DIM).transpose(2, 1, 0, 3)
            .reshape(H_LOC, P, NE * HEAD_DIM).astype(bf16))

    def prep_colmajor(w, nchunk):  # [K, M] -> [P, nchunk*M]: [p, c*M+m] = w[c*128+p, m]
        k, mm = w.shape
        return np.ascontiguousarray(
            w.reshape(nchunk, P, mm).transpose(1, 0, 2).reshape(P, nchunk * mm)
            .astype(bf16))

    f8np = ml_dtypes.float8_e4m3

    def prep_fchunk(w, nout, scale):  # [K, F] -> [F/128, P, (K/128)*128], fp8 x scale
        k, ff = w.shape
        nk = k // P
        ws = np.clip(w * scale, -240.0, 240.0)
        return np.ascontiguousarray(
            ws.reshape(nk, P, nout, P).transpose(2, 1, 0, 3)
            .reshape(nout, P, nk * P).astype(f8np))

    in_maps = []
    for c in range(N_CORES):
        b, t = divmod(c, TP)
        hs = slice(H_LOC * t, H_LOC * (t + 1))
        fs = slice(F_LOC * t, F_LOC * (t + 1))
        es = slice(ROWS_T * t, ROWS_T * (t + 1))
        xtb = np.ascontiguousarray(x[b].T)  # [E, S]
        in_maps.append(
            {
                "xb": np.ascontiguousarray(
                    xtb.astype(bf16).reshape(NE, P, NQB, QB)
                    .transpose(1, 2, 0, 3).reshape(P, NQB * NE * QB)),
                "xte": np.ascontiguousarray(xtb[es, :]),
                "wq": prep_qk(wq_f[:, hs, :]),
                "wk": prep_qk(wk_f[:, hs, :]),
                "wv": prep_colmajor(wv_f[:, hs, :].reshape(EMBED, HD), NE),
                "wout2": prep_colmajor(wout_f[:, es], NHD),
                "wg": prep_fchunk(wg_s[:, fs], NF, WG_SCALE),
                "wu": prep_fchunk(wu_s[:, fs], NF, WU_SCALE),
                "wd": prep_fchunk(w_down[fs, :], NE, WD_SCALE),
                "masks": np.ascontiguousarray(masks),
                "ones": np.ones((P, 1), bf16),
            }
        )
    return in_maps


def _install_profile_hook():
    import sys
    import types

    try:
        import antenv.axon_hooks  # noqa: F401

        return
    except ImportError:
        pass
    try:
        from trn_agent_boot.trn_boot import _ntff_profile_via_ctypes

        _hook = _ntff_profile_via_ctypes("/opt/axon/libaxon_pjrt.so")
        _mod = types.ModuleType("antenv.axon_hooks")
        _mod.get_axon_ntff_profile_hook = lambda: _hook
        sys.modules["antenv.axon_hooks"] = _mod
    except Exception:
        pass


def _run(nc, in_maps, trace=False, trace_cores=None):
    _install_profile_hook()
    from concourse.bass_utils import run_bass_kernel_spmd

    return run_bass_kernel_spmd(
        nc,
        in_maps,
        core_ids=list(range(N_CORES)),
        trace=trace,
        trace_cores=trace_cores,
    )


def kernel(**inputs):
    if "nc" not in _NC_CACHE:
        _NC_CACHE["nc"] = build_kernel()
    nc = _NC_CACHE["nc"]
    in_maps = _prep_core_inputs(inputs)
    res = _run(nc, in_maps)
    out = np.empty((BATCH, SEQ, EMBED), np.float32)
    for c in range(N_CORES):
        b, t = divmod(c, TP)
        out[b, :, ROWS_T * t:ROWS_T * (t + 1)] = res.results[c]["out"].T
    return out


if __name__ == "__main__":
    build_kernel()
    print("build ok")


# revision 25
# speedup vs baseline: 1.1211x; 1.0398x over previous
"""Trainium2 Bass kernel for a dense transformer block (RMSNorm->MHA->res, RMSNorm->SwiGLU-FFN->res).

Sharding over 8 NeuronCores: fsdp=2 (batch) x tp=4 (attention heads / FFN hidden).
Core 4*b + t handles batch b with TP-rank t (heads 4t..4t+3, FFN hidden cols 2048t..2048(t+1)).

All matmul operands are bf16 (full PE rate, FWL weight loads, half the SBUF/DMA
bytes); accumulation and the residual stream stay fp32.  On-device activations are
feature-major ([features, rows]) so matmuls chain without transposes.  Streamed
weights are pre-arranged on the host into per-partition-contiguous [128, X]
layouts so every weight DMA is a single >=2KB descriptor per partition.

Collective plan (per 4-core TP group, all payloads bf16 -- CCE adds in bf16):
  AG1(qb): AllGather of per-rank attention-head outputs ao [512(hd),512] for
           q-block qb -> [2048(hd),512].  Each rank then computes the FULL
           out-projection for its 512-row E-shard locally (same FLOPs as the
           head-sharded projection; an AllGather costs half a ReduceScatter on
           the wire and removes partial-sum staging).
  AG2(h):  AllGather of raw x2 E-shards [512,1024] -> [2048,1024] per row-half.
           rms2 runs locally AFTER the gather (no mean-square AllReduce).
  RS2(g):  ReduceScatter of FFN down partials [2048,512] -> [512,512] per
           512-row group.
Emission order gives every collective >=1 compute phase of slack before its
first consumer's DMA is issued, so engine sequencers never block on collective
semaphores.  Final output per core: [its 512 E-features, 2048 rows].
"""

import numpy as np

EMBED = 2048
HEADS = 16
HEAD_DIM = 128
FF_HID = 8192
BATCH = 2
SEQ = 2048
EPS = 1e-6

N_CORES = 8
TP = 4
GROUPS = [[0, 1, 2, 3], [4, 5, 6, 7]]
H_LOC = HEADS // TP          # 4 heads per core
F_LOC = FF_HID // TP         # 2048 ffn-hidden per core
ROWS = SEQ                   # 2048 rows per batch
ROWS_T = ROWS // TP          # 512 rows per tp-rank (E-shard size)
P = 128
NE = EMBED // P              # 16 embed chunks
NF = F_LOC // P              # 16 ffn chunks
NR = ROWS // P               # 16 row chunks
QB = 512                     # q-block / phase row count / matmul moving size
NQB = ROWS // QB             # 4 phases
RH = 1024                    # ffn row-half (AG2 granularity)
HD = H_LOC * HEAD_DIM        # 512 local head-dim rows
NHD = HEADS * HEAD_DIM // P  # 16 gathered hd chunks
INV_SQRT_D = float(1.0 / np.sqrt(HEAD_DIM))
# fp8(e4m3, max 240) FFN weight pre-scales keep the tiny (std~0.01) weights out
# of the subnormal range; the gelu input scale and the down-stage copy undo them.
WG_SCALE = 64.0
WU_SCALE = 16.0
WD_SCALE = 64.0
RS2_SCALE = 16.0  # fp8 RS2 payload carries 16x the true FFN partial

_NC_CACHE = {}


def build_kernel():
    import concourse.mybir as mybir
    import concourse.tile as tile
    from concourse import bacc

    f32 = mybir.dt.float32
    bf16 = mybir.dt.bfloat16

    nc = bacc.Bacc("TRN2", target_bir_lowering=False, debug=False, num_devices=N_CORES)

    io = {}
    # xb is host-pre-arranged [128, qb*8192 + e*512 + q] so each phase load is
    # one 16KB-contiguous descriptor per partition
    io["xb"] = nc.dram_tensor("xb", [P, NQB * NE * QB], bf16, kind="ExternalInput").ap()
    io["xte"] = nc.dram_tensor("xte", [ROWS_T, ROWS], f32, kind="ExternalInput").ap()
    io["wq"] = nc.dram_tensor("wq", [H_LOC, P, NE * HEAD_DIM], bf16, kind="ExternalInput").ap()
    io["wk"] = nc.dram_tensor("wk", [H_LOC, P, NE * HEAD_DIM], bf16, kind="ExternalInput").ap()
    io["wv"] = nc.dram_tensor("wv", [P, NE * HD], bf16, kind="ExternalInput").ap()
    io["wout2"] = nc.dram_tensor("wout2", [P, NHD * ROWS_T], bf16, kind="ExternalInput").ap()
    f8 = mybir.dt.float8e4
    io["wg"] = nc.dram_tensor("wg", [NF, P, NE * P], f8, kind="ExternalInput").ap()
    io["wu"] = nc.dram_tensor("wu", [NF, P, NE * P], f8, kind="ExternalInput").ap()
    io["wd"] = nc.dram_tensor("wd", [NE, P, NF * P], f8, kind="ExternalInput").ap()
    io["masks"] = nc.dram_tensor("masks", [P, QB + 3 * P], bf16, kind="ExternalInput").ap()
    io["ones"] = nc.dram_tensor("ones", [P, 1], bf16, kind="ExternalInput").ap()
    io["out"] = nc.dram_tensor("out", [ROWS_T, ROWS], f32, kind="ExternalOutput").ap()

    with tile.TileContext(nc) as tc:
        _emit(tc, nc, io)
    nc.compile()
    return nc


def _emit(tc, nc, io):
    from contextlib import ExitStack

    import concourse.mybir as mybir

    f32 = mybir.dt.float32
    bf16 = mybir.dt.bfloat16
    f8 = mybir.dt.float8e4
    DR = mybir.MatmulPerfMode.DoubleRow
    AF = mybir.ActivationFunctionType

    xb_in, xte, wq, wk, wv = io["xb"], io["xte"], io["wq"], io["wk"], io["wv"]
    ones_in = io["ones"]
    wout2, wg, wu, wd, masks = io["wout2"], io["wg"], io["wu"], io["wd"], io["masks"]
    out_ext = io["out"]

    def r3(ap2d, cols=None):
        """[(o p), q] dram view -> [p, o, q]; optionally slice columns first."""
        v = ap2d if cols is None else ap2d[:, cols]
        return v.rearrange("(o p) q -> p o q", p=P)

    ctx = ExitStack()
    with ctx:
        consts = ctx.enter_context(tc.tile_pool(name="consts", bufs=1))
        dram = ctx.enter_context(tc.tile_pool(name="dram", bufs=1, space="DRAM"))
        psum = ctx.enter_context(tc.tile_pool(name="psum", bufs=2, space="PSUM"))

        ones_sb = consts.tile([P, 1], bf16)
        nc.sync.dma_start(ones_sb[:], ones_in[:])
        eps_sb = consts.tile([1, 1], f32)
        nc.vector.memset(eps_sb[:], EPS)
        mask_sb = consts.tile([P, QB + 3 * P], bf16)
        nc.sync.dma_start(mask_sb[:], masks[:])
        wo_sb = consts.tile([P, NHD, ROWS_T], bf16)  # DMA deferred off the startup path

        # one dram tile per collective slice — a shared tensor would add false
        # whole-tensor deps (e.g. an ag2_out[0] reader waiting on AG2(1)'s write)
        ag1_in = [dram.tile([HD, QB], bf16, name=f"ag1i{i}") for i in range(NQB)]
        ag1_out = [dram.tile([HEADS * HEAD_DIM, QB], bf16, name=f"ag1o{i}") for i in range(NQB)]
        ag2_in = [dram.tile([ROWS_T, QB], bf16, name=f"ag2i{i}") for i in range(NQB)]
        ag2_out = [dram.tile([EMBED, QB], bf16, name=f"ag2o{i}") for i in range(NQB)]
        rs2_in = [dram.tile([EMBED, QB], f8, name=f"rs2i{i}") for i in range(NQB)]
        rs2_out = [dram.tile([ROWS_T, QB], f8, name=f"rs2o{i}") for i in range(NQB)]
        x2_scr = dram.tile([ROWS_T, ROWS], f32)

        with tc.tile_pool(name="s12", bufs=1) as s12:
            k_store = s12.tile([P, H_LOC, ROWS], bf16)
            v_store = s12.tile([P, NR, H_LOC, HEAD_DIM], bf16)

            tiles = {}

            def big16(name):
                """3-slot rotation shared by xq(0..3), ago(0..3) and n2(0..3) —
                16KB/partition tiles whose live ranges interleave exactly 3-deep."""
                return s12.tile([P, NE, QB], bf16, tag="big16", bufs=3, name=name)

            # ---------- stage 1+2 pieces ----------
            def emit_xb_dma(qb):
                xq = big16(f"xq{qb}")
                nc.sync.dma_start(xq[:].rearrange("p a b -> p (a b)"),
                                  xb_in[:, qb * NE * QB:(qb + 1) * NE * QB])
                tiles[("xq", qb)] = xq

            def emit_sq_ms(qb):
                ms = psum.tile([1, QB], f32, tag="acc", bufs=2, name=f"ms{qb}")
                xq = tiles[("xq", qb)]
                for e in range(NE):
                    sq = s12.tile([P, QB], bf16, tag="sq", bufs=2)
                    nc.vector.tensor_mul(sq[:], xq[:, e, :], xq[:, e, :])
                    nc.tensor.matmul(ms[:], ones_sb[:], sq[:],
                                     start=(e == 0), stop=(e == NE - 1))
                tiles[("ms", qb)] = ms

            def emit_norm_tail(qb):
                ms = tiles.pop(("ms", qb))
                rsq = s12.tile([1, QB], f32, tag="rsq", bufs=1)
                nc.scalar.activation(rsq[:], ms[:], AF.Sqrt, bias=eps_sb[:], scale=1.0 / EMBED)
                rsq_i = s12.tile([1, QB], f32, tag="rsqi", bufs=1)
                nc.vector.reciprocal(rsq_i[:], rsq[:])
                bc = s12.tile([P, QB], f32, tag="bc", bufs=2)
                nc.gpsimd.partition_broadcast(bc[:], rsq_i[:])
                xq = tiles[("xq", qb)]
                for e in range(NE):
                    nc.vector.tensor_mul(xq[:, e, :], xq[:, e, :], bc[:])

            def emit_qkv(qb):
                xq = tiles.pop(("xq", qb))
                cols = slice(qb * QB, (qb + 1) * QB)
                q_ph = s12.tile([P, H_LOC, QB], bf16, tag="q_ph", bufs=1, name=f"q{qb}")
                for h in range(H_LOC):
                    wq_sb = s12.tile([P, NE * HEAD_DIM], bf16, tag="wqk", bufs=3)
                    nc.sync.dma_start(wq_sb[:], wq[h])
                    wk_sb = s12.tile([P, NE * HEAD_DIM], bf16, tag="wqk", bufs=3)
                    nc.sync.dma_start(wk_sb[:], wk[h])
                    q_ps = psum.tile([P, QB], f32, tag="pC", bufs=2)
                    for e in range(NE):
                        nc.tensor.matmul(q_ps[:], wq_sb[:, e * P:(e + 1) * P],
                                         xq[:, e, :],
                                         start=(e == 0), stop=(e == NE - 1))
                    nc.vector.tensor_copy(q_ph[:, h, :], q_ps[:])
                    k_ps = psum.tile([P, QB], f32, tag="pC", bufs=2)
                    for e in range(NE):
                        nc.tensor.matmul(k_ps[:], wk_sb[:, e * P:(e + 1) * P],
                                         xq[:, e, :],
                                         start=(e == 0), stop=(e == NE - 1))
                    nc.scalar.activation(k_store[:, h, cols], k_ps[:], AF.Copy)
                # v: e-outer with wv streamed; 4 row-chunk accumulators borrow
                # the pA/pB PSUM slots (idle between attention blocks)
                v_ps = [
                    psum.tile([P, HD], f32, tag=t, bufs=2, name=f"v_ps{i}")
                    for i, t in enumerate(("pA", "pA", "pB", "pB"))
                ]
                for e in range(NE):
                    wv_e = s12.tile([P, HD], bf16, tag="wv_e", bufs=3)
                    nc.sync.dma_start(wv_e[:], wv[:, e * HD:(e + 1) * HD])
                    for rc in range(QB // P):
                        nc.tensor.matmul(v_ps[rc][:], xq[:, e, rc * P:(rc + 1) * P],
                                         wv_e[:],
                                         start=(e == 0), stop=(e == NE - 1))
                for rc in range(QB // P):
                    rcg = qb * (QB // P) + rc
                    nc.vector.tensor_copy(
                        v_store[:, rcg].rearrange("p h d -> p (h d)"), v_ps[rc][:])
                return q_ph

            def emit_attention(qb, q_ph):
                """Head-PAIR interleaved: two heads share the kc loop so the PE
                has ~6 matmuls in flight per exp/mask round-trip instead of 3."""
                ao_ph = s12.tile([P, H_LOC, QB], bf16, tag="ao_ph", bufs=1, name=f"ao{qb}")
                nk = (qb + 1) * (QB // P)
                for hp in range(0, H_LOC, 2):
                    pair = (hp, hp + 1)
                    pv_ps = {h: psum.tile([P, QB], f32, tag="pB", bufs=2,
                                          name=f"pv{qb}_{h}") for h in pair}
                    sum_ps = {h: psum.tile([1, QB], f32, tag="acc", bufs=2,
                                           name=f"sm{qb}_{h}") for h in pair}
                    lg_tiles = {}

                    def emit_lg(h, kc):
                        lg = psum.tile([P, QB], f32, tag="pA", bufs=2)
                        nc.tensor.matmul(
                            lg[:], k_store[:, h, kc * P:(kc + 1) * P],
                            q_ph[:, h, :], start=True, stop=True)
                        lg_tiles[(h, kc)] = lg

                    emit_lg(pair[0], 0)
                    emit_lg(pair[1], 0)
                    for kc in range(nk):
                        first, last = kc == 0, kc == nk - 1
                        j = kc - qb * (QB // P)
                        for h in pair:
                            lg = lg_tiles.pop((h, kc))
                            expt = s12.tile([P, QB], bf16, tag="expt", bufs=3)
                            nc.scalar.activation(expt[:], lg[:], AF.Exp,
                                                 scale=INV_SQRT_D)
                            if j >= 0:
                                off = (3 - j) * P
                                nc.vector.tensor_mul(expt[:], expt[:],
                                                     mask_sb[:, off:off + QB])
                            if kc + 1 < nk:
                                emit_lg(h, kc + 1)
                            nc.tensor.matmul(pv_ps[h][:], v_store[:, kc, h, :],
                                             expt[:], start=first, stop=last)
                            nc.tensor.matmul(sum_ps[h][:], ones_sb[:], expt[:],
                                             start=first, stop=last)
                    for h in pair:
                        rec = s12.tile([1, QB], f32, tag="rec", bufs=2)
                        nc.vector.reciprocal(rec[:], sum_ps[h][:])
                        rbc = s12.tile([P, QB], f32, tag="bc", bufs=2)
                        nc.gpsimd.partition_broadcast(rbc[:], rec[:])
                        nc.vector.tensor_mul(ao_ph[:, h, :], pv_ps[h][:], rbc[:])
                return ao_ph

            def emit_ao_stage(qb, ao_ph):
                nc.sync.dma_start(r3(ag1_in[qb]), ao_ph[:])
                nc.gpsimd.collective_compute(
                    "AllGather", mybir.AluOpType.bypass, replica_groups=GROUPS,
                    ins=[ag1_in[qb][:].opt()], outs=[ag1_out[qb][:].opt()],
                )

            def emit_ago_dma(qb, engine=None):
                ago = big16(f"ago{qb}")
                (engine or nc.sync).dma_start(ago[:], r3(ag1_out[qb]))
                tiles[("ago", qb)] = ago

            def emit_outproj(qb):
                """Full out-projection for my E-shard from gathered heads, then
                x2 = xte + proj; stage x2 (fp32->x2_scr, bf16->ag2_in)."""
                cols = slice(qb * QB, (qb + 1) * QB)
                ago = tiles.pop(("ago", qb))
                pr_ps = [
                    psum.tile([P, QB], f32, tag=t, bufs=2, name=f"pr{qb}_{i}")
                    for i, t in enumerate(("pA", "pA", "pB", "pB"))
                ]
                for c in range(NHD):
                    for e4 in range(H_LOC):
                        nc.tensor.matmul(pr_ps[e4][:],
                                         wo_sb[:, c, e4 * P:(e4 + 1) * P],
                                         ago[:, c, :],
                                         start=(c == 0), stop=(c == NHD - 1))
                for e4 in range(H_LOC):
                    xe_c = s12.tile([P, QB], f32, tag="xe", bufs=2)
                    nc.sync.dma_start(xe_c[:], r3(xte, cols)[:, e4, :])
                    x2_c = s12.tile([P, QB], f32, tag="x2", bufs=2)
                    nc.vector.tensor_add(x2_c[:], pr_ps[e4][:], xe_c[:])
                    nc.sync.dma_start(r3(x2_scr, cols)[:, e4, :], x2_c[:])
                    x2b_c = s12.tile([P, QB], bf16, tag="x2b", bufs=2)
                    nc.vector.tensor_copy(x2b_c[:], x2_c[:])
                    nc.sync.dma_start(r3(ag2_in[qb])[:, e4, :], x2b_c[:])

            def emit_ag2(qb):
                nc.gpsimd.collective_compute(
                    "AllGather", mybir.AluOpType.bypass, replica_groups=GROUPS,
                    ins=[ag2_in[qb][:].opt()], outs=[ag2_out[qb][:].opt()],
                )

            # ---------- stage 5 (FFN) + stage 6 pieces ----------
            def emit_n2_dma(g):
                n2 = big16(f"n2_{g}")
                nc.sync.dma_start(n2[:], r3(ag2_out[g]))
                tiles[("n2w", g)] = n2

            def emit_rms2(g):
                n2 = tiles.pop(("n2w", g))
                ms2 = psum.tile([1, QB], f32, tag="acc", bufs=2, name=f"ms2_{g}")
                for e in range(NE):
                    sq2 = s12.tile([P, QB], bf16, tag="sq", bufs=2)
                    nc.vector.tensor_mul(sq2[:], n2[:, e, :], n2[:, e, :])
                    nc.tensor.matmul(ms2[:], ones_sb[:], sq2[:],
                                     start=(e == 0), stop=(e == NE - 1))
                rsq2 = s12.tile([1, QB], f32, tag="rsq", bufs=1)
                nc.scalar.activation(rsq2[:], ms2[:], AF.Sqrt, bias=eps_sb[:],
                                     scale=1.0 / EMBED)
                rsq2_i = s12.tile([1, QB], f32, tag="rsqi", bufs=1)
                nc.vector.reciprocal(rsq2_i[:], rsq2[:])
                bc2 = s12.tile([P, QB], f32, tag="bc", bufs=2)
                nc.gpsimd.partition_broadcast(bc2[:], rsq2_i[:])
                n8 = s12.tile([P, NE, QB], f8, tag="n8", bufs=2, name=f"n8_{g}")
                for e in range(NE):
                    nc.vector.tensor_mul(n8[:, e, :], n2[:, e, :], bc2[:])
                tiles[("n8", g)] = n8

            def dr2(w_sb, e2):
                """[P, 256] fp8 slice -> [P, 2, 128] DoubleRow stationary view."""
                return w_sb[:, 2 * e2 * P:(2 * e2 + 2) * P].rearrange(
                    "p (two j) -> p two j", two=2)

            def emit_gateup(g):
                n8 = tiles[("n8", g)]
                act = s12.tile([P, NF, QB], f8, tag="act", bufs=2, name=f"act{g}")
                for f in range(NF):
                    wg_sb = s12.tile([P, NE * P], f8, tag="wgu", bufs=4)
                    nc.sync.dma_start(wg_sb[:], wg[f])
                    wu_sb = s12.tile([P, NE * P], f8, tag="wgu", bufs=4)
                    nc.sync.dma_start(wu_sb[:], wu[f])
                    g_ps = psum.tile([P, QB], f32, tag="pA", bufs=2)
                    for e2 in range(NE // 2):
                        nc.tensor.matmul(g_ps[:], dr2(wg_sb, e2),
                                         n8[:, 2 * e2:2 * e2 + 2, :],
                                         start=(e2 == 0), stop=(e2 == NE // 2 - 1),
                                         perf_mode=DR)
                    u_ps = psum.tile([P, QB], f32, tag="pB", bufs=2)
                    for e2 in range(NE // 2):
                        nc.tensor.matmul(u_ps[:], dr2(wu_sb, e2),
                                         n8[:, 2 * e2:2 * e2 + 2, :],
                                         start=(e2 == 0), stop=(e2 == NE // 2 - 1),
                                         perf_mode=DR)
                    gel = s12.tile([P, QB], bf16, tag="gel", bufs=2)
                    nc.scalar.activation(gel[:], g_ps[:], AF.Gelu_apprx_tanh,
                                         scale=1.0 / WG_SCALE)
                    nc.vector.tensor_mul(act[:, f, :], gel[:], u_ps[:])
                tiles[("act", g)] = act

            def emit_downs(g):
                act = tiles.pop(("act", g))
                tiles.pop(("n8", g), None)
                for e in range(NE):
                    wd_sb = s12.tile([P, NF * P], f8, tag="wd", bufs=2)
                    nc.sync.dma_start(wd_sb[:], wd[e])
                    d_ps = psum.tile([P, QB], f32, tag="pC", bufs=2)
                    for f2 in range(NF // 2):
                        nc.tensor.matmul(d_ps[:], dr2(wd_sb, f2),
                                         act[:, 2 * f2:2 * f2 + 2, :],
                                         start=(f2 == 0), stop=(f2 == NF // 2 - 1),
                                         perf_mode=DR)
                    d_sb = s12.tile([P, QB], f8, tag="dstage", bufs=2)
                    nc.scalar.activation(d_sb[:], d_ps[:], AF.Copy,
                                         scale=RS2_SCALE / (WU_SCALE * WD_SCALE))
                    nc.sync.dma_start(r3(rs2_in[g][e * P:(e + 1) * P, :]), d_sb[:])
                nc.gpsimd.collective_compute(
                    "ReduceScatter", mybir.AluOpType.add, replica_groups=GROUPS,
                    ins=[rs2_in[g][:].opt()], outs=[rs2_out[g][:].opt()],
                )

            def emit_stage6(g):
                cols = slice(g * QB, (g + 1) * QB)
                for e4 in range(H_LOC):
                    fsum = s12.tile([P, QB], f8, tag="fsum", bufs=1)
                    nc.sync.dma_start(fsum[:], r3(rs2_out[g])[:, e4, :])
                    fs32 = s12.tile([P, QB], f32, tag="fs32", bufs=1)
                    nc.scalar.activation(fs32[:], fsum[:], AF.Copy,
                                         scale=1.0 / RS2_SCALE)
                    x2r = s12.tile([P, QB], f32, tag="x2r", bufs=1)
                    nc.sync.dma_start(x2r[:], r3(x2_scr, cols)[:, e4, :])
                    fin = s12.tile([P, QB], f32, tag="fin", bufs=1)
                    nc.vector.tensor_add(fin[:], fs32[:], x2r[:])
                    nc.sync.dma_start(r3(out_ext, cols)[:, e4, :], fin[:])

            # ---------- schedule ----------
            # outproj(qb-1) is emitted AFTER attention(qb): the ago load's
            # AG1-completion wait sits on the (idle) sync queue during
            # attention, and the PE reaches outproj long after AG1 finished.
            emit_xb_dma(0)
            emit_sq_ms(0)
            emit_norm_tail(0)
            # outproj(qb-2) runs two phases after its AG1: every gather and
            # every gather-output load has >=1 full compute phase of slack.
            # AG2 fires per phase right after its outproj.
            for qb in range(NQB):
                q_ph = emit_qkv(qb)
                if qb + 1 < NQB:
                    emit_xb_dma(qb + 1)
                if qb == 1:
                    nc.sync.dma_start(wo_sb[:].rearrange("p a b -> p (a b)"), wout2[:])
                if qb >= 2:
                    emit_ago_dma(qb - 2)
                if qb == 3:
                    emit_ago_dma(2)
                    emit_n2_dma(0)  # loads during attention(3); AG2(0) done by then
                ao_ph = emit_attention(qb, q_ph)
                emit_ao_stage(qb, ao_ph)
                if qb >= 2:
                    emit_outproj(qb - 2)
                    emit_ag2(qb - 2)
                if qb + 1 < NQB:
                    emit_sq_ms(qb + 1)
                    emit_norm_tail(qb + 1)

            emit_outproj(2)
            emit_ag2(2)
            emit_rms2(0)
            emit_gateup(0)
            # ago(3) load issues on the scalar queue: its AG1(3) wait only
            # delays gelu issuance (not PE), and outproj(3) runs after
            # downs(0) when the load is long done.
            emit_ago_dma(3, engine=nc.scalar)
            emit_n2_dma(1)
            emit_downs(0)
            emit_outproj(3)
            emit_ag2(3)
            emit_rms2(1)
            emit_gateup(1)
            emit_downs(1)
            emit_n2_dma(2)
            emit_rms2(2)
            emit_gateup(2)
            emit_stage6(0)
            emit_downs(2)
            emit_n2_dma(3)
            emit_rms2(3)
            emit_gateup(3)
            emit_stage6(1)
            emit_downs(3)
            emit_stage6(2)
            emit_stage6(3)


# ============================ host side ============================


def _prep_core_inputs(inputs):
    """Shard + transpose + fold rms scales into weights; pre-arrange streamed
    weights into per-partition-contiguous [128, X] layouts. 8 in_maps."""
    import ml_dtypes

    bf16 = ml_dtypes.bfloat16

    x = np.asarray(inputs["x"], np.float32)          # [B, S, E]
    w_qkv = np.asarray(inputs["w_qkv"], np.float32)  # [E, H, 3D]
    w_out = np.asarray(inputs["w_out"], np.float32)  # [H, D, E]
    w_gate = np.asarray(inputs["w_gate"], np.float32)
    w_up = np.asarray(inputs["w_up"], np.float32)
    w_down = np.asarray(inputs["w_down"], np.float32)
    scale1 = np.asarray(inputs["scale1"], np.float32)
    scale2 = np.asarray(inputs["scale2"], np.float32)

    wqkv_s = w_qkv * scale1[:, None, None]
    wq_f = wqkv_s[:, :, 0:HEAD_DIM]
    wk_f = wqkv_s[:, :, HEAD_DIM:2 * HEAD_DIM]
    wv_f = wqkv_s[:, :, 2 * HEAD_DIM:3 * HEAD_DIM]
    wout_f = w_out.reshape(HEADS * HEAD_DIM, EMBED)
    wg_s = w_gate * scale2[:, None]
    wu_s = w_up * scale2[:, None]

    kp = np.arange(P)[:, None]
    m = np.arange(QB + 3 * P)[None, :]
    masks = (m >= kp + 3 * P).astype(bf16)  # mask_j = masks[:, (3-j)*128 : (3-j)*128+512]

    def prep_qk(w):  # [E, H_LOC, D] -> [H_LOC, P, NE*D]: [h,p,e*D+d] = w[e*128+p,h,d]
        return np.ascontiguousarray(
            w.reshape(NE, P, H_LOC, HEAD_DIM).transpose(2, 1, 0, 3)
            .reshape(H_LOC, P, NE * HEAD_DIM).astype(bf16))

    def prep_colmajor(w, nchunk):  # [K, M] -> [P, nchunk*M]: [p, c*M+m] = w[c*128+p, m]
        k, mm = w.shape
        return np.ascontiguousarray(
            w.reshape(nchunk, P, mm).transpose(1, 0, 2).reshape(P, nchunk * mm)
            .astype(bf16))

    f8np = ml_dtypes.float8_e4m3

    def prep_fchunk(w, nout, scale):  # [K, F] -> [F/128, P, (K/128)*128], fp8 x scale
        k, ff = w.shape
        nk = k // P
        ws = np.clip(w * scale, -240.0, 240.0)
        return np.ascontiguousarray(
            ws.reshape(nk, P, nout, P).transpose(2, 1, 0, 3)
            .reshape(nout, P, nk * P).astype(f8np))

    in_maps = []
    for c in range(N_CORES):
        b, t = divmod(c, TP)
        hs = slice(H_LOC * t, H_LOC * (t + 1))
        fs = slice(F_LOC * t, F_LOC * (t + 1))
        es = slice(ROWS_T * t, ROWS_T * (t + 1))
        xtb = np.ascontiguousarray(x[b].T)  # [E, S]
        in_maps.append(
            {
                "xb": np.ascontiguousarray(
                    xtb.astype(bf16).reshape(NE, P, NQB, QB)
                    .transpose(1, 2, 0, 3).reshape(P, NQB * NE * QB)),
                "xte": np.ascontiguousarray(xtb[es, :]),
                "wq": prep_qk(wq_f[:, hs, :]),
                "wk": prep_qk(wk_f[:, hs, :]),
                "wv": prep_colmajor(wv_f[:, hs, :].reshape(EMBED, HD), NE),
                "wout2": prep_colmajor(wout_f[:, es], NHD),
                "wg": prep_fchunk(wg_s[:, fs], NF, WG_SCALE),
                "wu": prep_fchunk(wu_s[:, fs], NF, WU_SCALE),
                "wd": prep_fchunk(w_down[fs, :], NE, WD_SCALE),
                "masks": np.ascontiguousarray(masks),
                "ones": np.ones((P, 1), bf16),
            }
        )
    return in_maps


def _install_profile_hook():
    import sys
    import types

    try:
        import antenv.axon_hooks  # noqa: F401

        return
    except ImportError:
        pass
    try:
        from trn_agent_boot.trn_boot import _ntff_profile_via_ctypes

        _hook = _ntff_profile_via_ctypes("/opt/axon/libaxon_pjrt.so")
        _mod = types.ModuleType("antenv.axon_hooks")
        _mod.get_axon_ntff_profile_hook = lambda: _hook
        sys.modules["antenv.axon_hooks"] = _mod
    except Exception:
        pass


def _run(nc, in_maps, trace=False, trace_cores=None):
    _install_profile_hook()
    from concourse.bass_utils import run_bass_kernel_spmd

    return run_bass_kernel_spmd(
        nc,
        in_maps,
        core_ids=list(range(N_CORES)),
        trace=trace,
        trace_cores=trace_cores,
    )


def kernel(**inputs):
    if "nc" not in _NC_CACHE:
        _NC_CACHE["nc"] = build_kernel()
    nc = _NC_CACHE["nc"]
    in_maps = _prep_core_inputs(inputs)
    res = _run(nc, in_maps)
    out = np.empty((BATCH, SEQ, EMBED), np.float32)
    for c in range(N_CORES):
        b, t = divmod(c, TP)
        out[b, :, ROWS_T * t:ROWS_T * (t + 1)] = res.results[c]["out"].T
    return out


if __name__ == "__main__":
    build_kernel()
    print("build ok")
